# revision 18
# baseline (speedup 1.0000x reference)
"""TRN2 Bass kernel for nn_BlockPermProduct.

The reference applies 9 probabilistic block-permutation mixing steps to each
row of x [65536, 1024]. Every step is linear in x, so the whole transform is
``out = x @ M^T`` for a 1024x1024 matrix M built on the host (float64) from
the tiny (9, 3) logits. Structure analysis (block sparsity, hierarchical
ranks, Monarch tests) shows M sits exactly at the low-rank break-even point
at every scale, so a dense matmul is the right device algorithm; the wins
come from engineering it to the PE streaming floor:

  - bf16 end to end: x is pre-transposed AND cast to bf16 on the host, so
    the device does ZERO transposes (the old kernel burned a third of PE
    time on them); HBM traffic halves; FWL makes weight loads 2x faster.
  - out^T formulation: psum[i, r] = sum_j MT[j_block, i_block]^T @ xT[j, r]
    with the 64 constant 128x128 MT sub-blocks as stationary weights and
    xT chunks as N=512 moving operands. Weight loads are data-independent
    and hide under the 512-cycle streams.
  - i-outer / j-inner loop: each output chunk's PSUM bank drains (Vector
    and Scalar engines alternate on the fp32->bf16 copies) while the next
    chunk's matmuls stream -> no PE bubbles.
  - per-group 1 MiB DMAs (in and out), triple-buffered; warmup matmuls
    fill the head DMA wait so HAM unthrottles before the real stream; the
    last group's writeout is split so the tail is copy + 512 KiB.

The trace shows a gapless 1024-matmul stream at 215.9 ns each (= N/2.4GHz
+ NX overhead, the warm PE floor), ~221 us busy of ~245 us total; the rest
is fixed NEFF preamble/postamble and the head DMA fill. Host un-transposes
and casts the bf16 out^T back to fp32. Measured rel err 4.2e-3 (tolerance
2e-2). Sharding: pure data parallel over batch, 8 cores, no communication.

Run-to-run note: occasional runs land in a ~292 us state (PE at 2.0 GHz,
P0 power throttle) -- environmental, not kernel-dependent.
"""

import numpy as np
import ml_dtypes
from contextlib import ExitStack

import concourse.bass as bass
import concourse.bacc as bacc
import concourse.mybir as mybir
import concourse.tile as tile
from concourse.bass_utils import run_bass_kernel_spmd

BATCH = 65536
SIZE = 1024
N_CORES = 8
ROWS_PER_CORE = BATCH // N_CORES  # 8192
P = 128
N_CHUNK = SIZE // P  # 8
GW = 512  # rows ("r" columns of xT) per group
N_GROUPS = ROWS_PER_CORE // GW  # 16
HALF = 512

F32 = mybir.dt.float32
BF16 = mybir.dt.bfloat16
NP_BF16 = np.dtype(ml_dtypes.bfloat16)

MATMUL_MODE = "bf16_xt"

TRACE = False
TRACE_KWARGS = {}
LAST_RESULTS = None

_NC_CACHE = {}


def _transform64(y, logits):
    """Float64 port of the reference transform, applied to rows of y."""
    m = 10
    sizes = [SIZE >> i for i in range(m - 1)][::-1]  # [4, 8, ..., 1024]
    out = y
    for i in range(m - 2, -1, -1):
        n = sizes[i]
        p = 1.0 / (1.0 + np.exp(-logits[i].astype(np.float64)))
        z = out.reshape(-1, n)
        sep = z.reshape(-1, n // 2, 2).transpose(0, 2, 1).reshape(-1, n)
        z = (1 - p[0]) * z + p[0] * sep
        h = n // 2
        first = (1 - p[1]) * z[:, :h] + p[1] * z[:, h - 1::-1]
        second = (1 - p[2]) * z[:, h:] + p[2] * z[:, : h - 1 : -1]
        out = np.concatenate([first, second], axis=1).reshape(out.shape)
    return out


def _build_mt(logits):
    """M^T [1024, 1024] fp32: row j = transform(e_j), so MT[j, i] = M[i, j]."""
    eye = np.eye(SIZE, dtype=np.float64)
    mt = _transform64(eye, logits)
    return np.ascontiguousarray(mt.astype(np.float32))


def _build_bass():
    nc = bacc.Bacc("TRN2", target_bir_lowering=False, debug=False)
    xt = nc.dram_tensor("xt", [SIZE, ROWS_PER_CORE], BF16, kind="ExternalInput").ap()
    mt = nc.dram_tensor("mt", [SIZE, SIZE], BF16, kind="ExternalInput").ap()
    outt = nc.dram_tensor(
        "outt", [SIZE, ROWS_PER_CORE], BF16, kind="ExternalOutput"
    ).ap()

    with tile.TileContext(nc) as tc, ExitStack() as ctx:
        const = ctx.enter_context(tc.tile_pool(name="const", bufs=1))
        xpool = ctx.enter_context(tc.tile_pool(name="xin", bufs=3))
        opool = ctx.enter_context(tc.tile_pool(name="osb", bufs=3))
        pso = ctx.enter_context(tc.tile_pool(name="pso", bufs=4, space="PSUM"))

        # The host uploads mt retiled by OUTPUT chunk: DRAM rows
        # [i*128:(i+1)*128] hold mtcol[i][p, (j, n)] = MT[j*128+p, i*128+n],
        # so output chunk i of group 0 depends only on its own 256 KiB tile
        # (the i=0 loop starts after mtcol[0] + xin0 instead of the full
        # 2 MiB of weights). mtcol[0] loads first: the HAM-warmup matmuls
        # below depend only on it, so the PE starts ~2us in and is at
        # 2.4 GHz by the time the real stream begins.
        mts = []
        t0 = const.tile([P, SIZE], BF16, tag="mt0")
        nc.sync.dma_start(t0[:], mt[0:P, :])
        mts.append(t0)

        # First group's xT load goes ahead of the remaining M^T constants
        # so the PE isn't queued behind 2 MiB of weights.
        xin0 = xpool.tile([P, N_CHUNK * GW], BF16, tag="xin")
        nc.sync.dma_start(
            xin0[:].rearrange("p (c r) -> p c r", r=GW),
            xt[:, 0:GW].rearrange("(c p) r -> p c r", p=P),
        )

        # The remaining weight tiles load via the Scalar engine's HWDGE ring
        # so they don't steal SDMA round-robin attention from the critical
        # xin0 + mtcol[0] transfers on the Sync ring.
        for i in range(1, N_CHUNK):
            t = const.tile([P, SIZE], BF16, tag=f"mt{i}")
            nc.scalar.dma_start(t[:], mt[i * P : (i + 1) * P, :])
            mts.append(t)

        # Warmup matmuls on a memset tile: no DMA dependency, so the PE
        # starts right after the NEFF preamble (~7.4us) and stays busy until
        # group 0's data lands (~13.5us). ~8 cold MMs unthrottle HAM
        # (1.2 -> 2.4 GHz); the count is tuned to end at data-ready.
        wsrc = const.tile([P, GW], BF16, tag="wsrc")
        nc.vector.memset(wsrc[:], 0.0)
        wpo = pso.tile([P, GW], F32, tag="po")
        for _ in range(17):
            nc.tensor.matmul(
                wpo[:], wsrc[:, 0:P], wsrc[:], start=True, stop=True
            )

        for g in range(N_GROUPS):
            r0 = g * GW
            if g == 0:
                xin = xin0
            else:
                xin = xpool.tile([P, N_CHUNK * GW], BF16, tag="xin")
                nc.sync.dma_start(
                    xin[:].rearrange("p (c r) -> p c r", r=GW),
                    xt[:, r0 : r0 + GW].rearrange("(c p) r -> p c r", p=P),
                )
            if g == N_GROUPS - 1:
                otail = [
                    opool.tile([P, 2 * GW], BF16, tag=f"osbT{q}", name=f"osbT{q}")
                    for q in range(4)
                ]
                osb = None
            else:
                osb = opool.tile([P, N_CHUNK * GW], BF16, tag="osb")

            for i in range(N_CHUNK):
                po = pso.tile([P, GW], F32, tag="po")
                for j in range(N_CHUNK):
                    nc.tensor.matmul(
                        po[:],
                        mts[i][:, j * P : (j + 1) * P],
                        xin[:, j * GW : (j + 1) * GW],
                        start=(j == 0),
                        stop=(j == N_CHUNK - 1),
                    )
                # Alternate PSUM->SBUF (fp32->bf16) copies across engines.
                if g == N_GROUPS - 1:
                    # Last group: four pair tiles, each DMA'd as soon as its
                    # 2 chunks are copied, so the kernel tail is one short
                    # copy + one 256 KiB DMA instead of copy + 1 MiB DMA.
                    oh = otail[i // 2]
                    dst = oh[:, (i % 2) * GW : (i % 2 + 1) * GW]
                else:
                    dst = osb[:, i * GW : (i + 1) * GW]
                if g == N_GROUPS - 1 and i == N_CHUNK - 1:
                    # Split the very last copy across both engines to halve
                    # its latency on the kernel's critical path.
                    nc.vector.tensor_copy(dst[:, 0 : GW // 2], po[:, 0 : GW // 2])
                    nc.scalar.copy(dst[:, GW // 2 : GW], po[:, GW // 2 : GW])
                elif i % 2 == 0:
                    nc.vector.tensor_copy(dst, po[:])
                else:
                    nc.scalar.copy(dst, po[:])
                if g == N_GROUPS - 1 and i % 2 == 1:
                    h = i // 2
                    nc.sync.dma_start(
                        outt[h * 2 * P : (h + 1) * 2 * P, r0 : r0 + GW]
                        .rearrange("(c p) r -> p c r", p=P),
                        otail[h][:].rearrange("p (c r) -> p c r", r=GW),
                    )
            if g < N_GROUPS - 1:
                nc.sync.dma_start(
                    outt[:, r0 : r0 + GW].rearrange("(c p) r -> p c r", p=P),
                    osb[:].rearrange("p (c r) -> p c r", r=GW),
                )

    nc.compile()
    return nc


def _get_nc():
    key = MATMUL_MODE
    if key not in _NC_CACHE:
        _NC_CACHE[key] = _build_bass()
    return _NC_CACHE[key]


def kernel(x, logits):
    x = np.asarray(x)
    logits = np.asarray(logits)
    assert x.shape == (BATCH, SIZE)

    mtf = _build_mt(logits)
    # Retile by output chunk (see _build_bass): row block i holds
    # mtcol[i][p, (j, n)] = MT[j*128+p, i*128+n].
    mt = np.ascontiguousarray(
        mtf.reshape(N_CHUNK, P, N_CHUNK, P).transpose(2, 1, 0, 3).reshape(SIZE, SIZE)
    ).astype(NP_BF16)
    nc = _get_nc()

    in_maps = []
    for i in range(N_CORES):
        xc = x[i * ROWS_PER_CORE : (i + 1) * ROWS_PER_CORE]
        xtc = np.ascontiguousarray(xc.T.astype(NP_BF16))
        in_maps.append({"xt": xtc, "mt": mt})

    kwargs = dict(TRACE_KWARGS)
    if TRACE:
        kwargs.setdefault("trace", True)
        kwargs.setdefault("trace_cores", [0])
    res = run_bass_kernel_spmd(nc, in_maps, core_ids=list(range(N_CORES)), **kwargs)
    global LAST_RESULTS
    LAST_RESULTS = res
    return np.concatenate(
        [res.results[i]["outt"].T.astype(np.float32) for i in range(N_CORES)], axis=0
    )


# revision 20
# speedup vs baseline: 1.0163x; 1.0163x over previous
"""TRN2 Bass kernel for nn_BlockPermProduct.

The reference applies 9 probabilistic block-permutation mixing steps to each
row of x [65536, 1024]. Every step is linear in x, so the whole transform is
``out = x @ M^T`` for a 1024x1024 matrix M built on the host (float64) from
the tiny (9, 3) logits. Structure analysis (block sparsity, hierarchical
ranks, Monarch tests) shows M sits exactly at the low-rank break-even point
at every scale, so a dense matmul is the right device algorithm; the wins
come from engineering it to the PE streaming floor:

  - bf16 end to end: x is pre-transposed AND cast to bf16 on the host, so
    the device does ZERO transposes (the old kernel burned a third of PE
    time on them); HBM traffic halves; FWL makes weight loads 2x faster.
  - out^T formulation: psum[i, r] = sum_j MT[j_block, i_block]^T @ xT[j, r]
    with the 64 constant 128x128 MT sub-blocks as stationary weights and
    xT chunks as N=512 moving operands. Weight loads are data-independent
    and hide under the 512-cycle streams.
  - i-outer / j-inner loop: each output chunk's PSUM bank drains (Vector
    and Scalar engines alternate on the fp32->bf16 copies) while the next
    chunk's matmuls stream -> no PE bubbles.
  - per-group 1 MiB DMAs (in and out), triple-buffered; warmup matmuls
    fill the head DMA wait so HAM unthrottles before the real stream; the
    last group's writeout is split so the tail is copy + 512 KiB.

The trace shows a gapless 1024-matmul stream at 215.9 ns each (= N/2.4GHz
+ NX overhead, the warm PE floor), ~221 us busy of ~245 us total; the rest
is fixed NEFF preamble/postamble and the head DMA fill. Host un-transposes
and casts the bf16 out^T back to fp32. Measured rel err 4.2e-3 (tolerance
2e-2). Sharding: pure data parallel over batch, 8 cores, no communication.

Run-to-run note: occasional runs land in a ~292 us state (PE at 2.0 GHz,
P0 power throttle) -- environmental, not kernel-dependent.
"""

import numpy as np
import ml_dtypes
from contextlib import ExitStack

import concourse.bass as bass
import concourse.bacc as bacc
import concourse.mybir as mybir
import concourse.tile as tile
from concourse.bass_utils import run_bass_kernel_spmd

BATCH = 65536
SIZE = 1024
N_CORES = 8
ROWS_PER_CORE = BATCH // N_CORES  # 8192
P = 128
N_CHUNK = SIZE // P  # 8
GW = 512  # rows ("r" columns of xT) per group
N_GROUPS = ROWS_PER_CORE // GW  # 16
HALF = 512

F32 = mybir.dt.float32
BF16 = mybir.dt.bfloat16
NP_BF16 = np.dtype(ml_dtypes.bfloat16)

MATMUL_MODE = "bf16_xt"

TRACE = False
TRACE_KWARGS = {}
LAST_RESULTS = None

_NC_CACHE = {}


def _transform64(y, logits):
    """Float64 port of the reference transform, applied to rows of y."""
    m = 10
    sizes = [SIZE >> i for i in range(m - 1)][::-1]  # [4, 8, ..., 1024]
    out = y
    for i in range(m - 2, -1, -1):
        n = sizes[i]
        p = 1.0 / (1.0 + np.exp(-logits[i].astype(np.float64)))
        z = out.reshape(-1, n)
        sep = z.reshape(-1, n // 2, 2).transpose(0, 2, 1).reshape(-1, n)
        z = (1 - p[0]) * z + p[0] * sep
        h = n // 2
        first = (1 - p[1]) * z[:, :h] + p[1] * z[:, h - 1::-1]
        second = (1 - p[2]) * z[:, h:] + p[2] * z[:, : h - 1 : -1]
        out = np.concatenate([first, second], axis=1).reshape(out.shape)
    return out


def _build_mt(logits):
    """M^T [1024, 1024] fp32: row j = transform(e_j), so MT[j, i] = M[i, j]."""
    eye = np.eye(SIZE, dtype=np.float64)
    mt = _transform64(eye, logits)
    return np.ascontiguousarray(mt.astype(np.float32))


def _build_bass():
    nc = bacc.Bacc("TRN2", target_bir_lowering=False, debug=False)
    xt = nc.dram_tensor("xt", [SIZE, ROWS_PER_CORE], BF16, kind="ExternalInput").ap()
    mt = nc.dram_tensor("mt", [SIZE, SIZE], BF16, kind="ExternalInput").ap()
    outt = nc.dram_tensor(
        "outt", [SIZE, ROWS_PER_CORE], BF16, kind="ExternalOutput"
    ).ap()

    with tile.TileContext(nc) as tc, ExitStack() as ctx:
        const = ctx.enter_context(tc.tile_pool(name="const", bufs=1))
        xpool = ctx.enter_context(tc.tile_pool(name="xin", bufs=3))
        opool = ctx.enter_context(tc.tile_pool(name="osb", bufs=3))
        pso = ctx.enter_context(tc.tile_pool(name="pso", bufs=4, space="PSUM"))

        # The host uploads mt retiled by OUTPUT chunk: DRAM rows
        # [i*128:(i+1)*128] hold mtcol[i][p, (j, n)] = MT[j*128+p, i*128+n],
        # so output chunk i of group 0 depends only on its own 256 KiB tile
        # (the i=0 loop starts after mtcol[0] + xin0 instead of the full
        # 2 MiB of weights). mtcol[0] loads first: the HAM-warmup matmuls
        # below depend only on it, so the PE starts ~2us in and is at
        # 2.4 GHz by the time the real stream begins.
        mts = []
        t0 = const.tile([P, SIZE], BF16, tag="mt0")
        nc.sync.dma_start(t0[:], mt[0:P, :])
        mts.append(t0)

        # First group's xT load goes ahead of the remaining M^T constants
        # so the PE isn't queued behind 2 MiB of weights.
        xin0 = xpool.tile([P, N_CHUNK * GW], BF16, tag="xin")
        nc.sync.dma_start(
            xin0[:].rearrange("p (c r) -> p c r", r=GW),
            xt[:, 0:GW].rearrange("(c p) r -> p c r", p=P),
        )

        for i in range(1, N_CHUNK):
            t = const.tile([P, SIZE], BF16, tag=f"mt{i}")
            nc.sync.dma_start(t[:], mt[i * P : (i + 1) * P, :])
            mts.append(t)

        # Warmup matmuls on a memset tile: no DMA dependency, so the PE
        # starts right after the NEFF preamble (~7.4us) and stays busy until
        # group 0's data lands (~13.5us). ~8 cold MMs unthrottle HAM
        # (1.2 -> 2.4 GHz); the count is tuned to end at data-ready.
        wsrc = const.tile([P, GW], BF16, tag="wsrc")
        nc.vector.memset(wsrc[:], 0.0)
        wpo = pso.tile([P, GW], F32, tag="po")
        for _ in range(20):
            nc.tensor.matmul(
                wpo[:], wsrc[:, 0:P], wsrc[:], start=True, stop=True
            )

        for g in range(N_GROUPS):
            r0 = g * GW
            if g == 0:
                xin = xin0
            else:
                xin = xpool.tile([P, N_CHUNK * GW], BF16, tag="xin")
                nc.sync.dma_start(
                    xin[:].rearrange("p (c r) -> p c r", r=GW),
                    xt[:, r0 : r0 + GW].rearrange("(c p) r -> p c r", p=P),
                )
            if g == N_GROUPS - 1:
                otail = [
                    opool.tile([P, 2 * GW], BF16, tag=f"osbT{q}", name=f"osbT{q}")
                    for q in range(4)
                ]
                osb = None
            else:
                osb = opool.tile([P, N_CHUNK * GW], BF16, tag="osb")

            for i in range(N_CHUNK):
                po = pso.tile([P, GW], F32, tag="po")
                for j in range(N_CHUNK):
                    nc.tensor.matmul(
                        po[:],
                        mts[i][:, j * P : (j + 1) * P],
                        xin[:, j * GW : (j + 1) * GW],
                        start=(j == 0),
                        stop=(j == N_CHUNK - 1),
                    )
                # Alternate PSUM->SBUF (fp32->bf16) copies across engines.
                if g == N_GROUPS - 1:
                    # Last group: four pair tiles, each DMA'd as soon as its
                    # 2 chunks are copied, so the kernel tail is one short
                    # copy + one 256 KiB DMA instead of copy + 1 MiB DMA.
                    oh = otail[i // 2]
                    dst = oh[:, (i % 2) * GW : (i % 2 + 1) * GW]
                else:
                    dst = osb[:, i * GW : (i + 1) * GW]
                if g == N_GROUPS - 1 and i == N_CHUNK - 1:
                    # Split the very last copy across both engines to halve
                    # its latency on the kernel's critical path.
                    nc.vector.tensor_copy(dst[:, 0 : GW // 2], po[:, 0 : GW // 2])
                    nc.scalar.copy(dst[:, GW // 2 : GW], po[:, GW // 2 : GW])
                elif i % 2 == 0:
                    nc.vector.tensor_copy(dst, po[:])
                else:
                    nc.scalar.copy(dst, po[:])
                if g == N_GROUPS - 1 and i % 2 == 1:
                    h = i // 2
                    nc.sync.dma_start(
                        outt[h * 2 * P : (h + 1) * 2 * P, r0 : r0 + GW]
                        .rearrange("(c p) r -> p c r", p=P),
                        otail[h][:].rearrange("p (c r) -> p c r", r=GW),
                    )
            if g < N_GROUPS - 1:
                nc.sync.dma_start(
                    outt[:, r0 : r0 + GW].rearrange("(c p) r -> p c r", p=P),
                    osb[:].rearrange("p (c r) -> p c r", r=GW),
                )

    nc.compile()
    return nc


def _get_nc():
    key = MATMUL_MODE
    if key not in _NC_CACHE:
        _NC_CACHE[key] = _build_bass()
    return _NC_CACHE[key]


def kernel(x, logits):
    x = np.asarray(x)
    logits = np.asarray(logits)
    assert x.shape == (BATCH, SIZE)

    mtf = _build_mt(logits)
    # Retile by output chunk (see _build_bass): row block i holds
    # mtcol[i][p, (j, n)] = MT[j*128+p, i*128+n].
    mt = np.ascontiguousarray(
        mtf.reshape(N_CHUNK, P, N_CHUNK, P).transpose(2, 1, 0, 3).reshape(SIZE, SIZE)
    ).astype(NP_BF16)
    nc = _get_nc()

    in_maps = []
    for i in range(N_CORES):
        xc = x[i * ROWS_PER_CORE : (i + 1) * ROWS_PER_CORE]
        xtc = np.ascontiguousarray(xc.T.astype(NP_BF16))
        in_maps.append({"xt": xtc, "mt": mt})

    kwargs = dict(TRACE_KWARGS)
    if TRACE:
        kwargs.setdefault("trace", True)
        kwargs.setdefault("trace_cores", [0])
    res = run_bass_kernel_spmd(nc, in_maps, core_ids=list(range(N_CORES)), **kwargs)
    global LAST_RESULTS
    LAST_RESULTS = res
    return np.concatenate(
        [res.results[i]["outt"].T.astype(np.float32) for i in range(N_CORES)], axis=0
    )
